# revision 19
# baseline (speedup 1.0000x reference)
"""Expert-parallel grouped matmul (MoE BatchLinear) for 8 Trainium2 NeuronCores.

Problem: y[t] = x[t] @ W[g(t)] where tokens are grouped contiguously by expert
g (G=64 experts, counts given at runtime). Sharding: expert-parallel — core c
owns experts [8c, 8c+8) and the contiguous token rows routed to them. The
"all-to-all" is done host-side: kernel() receives full inputs, slices/pads
per-core token blocks, and scatters per-core outputs back.

Device kernel (SPMD, one program on 8 cores):
  for each local expert e (8 per core):
    xT_e resident in SBUF as [128ki, 8ko, Te] (host pre-transposed)
    for each NQW-wide n-slab of W_e (slab [128ki, 8ko, NQW], 4 bufs):
      for each 128-token m-tile:
        8 k-steps x NB matmuls (N=512) accumulate into NB PSUM banks
        DVE copy PSUM -> SBUF staging, DMA staging -> y (scalar-engine ring)

All DRAM layouts are chosen so every DMA reads/writes fully-contiguous
per-partition runs: W as [e, q, ki, ko, n], xT as per-expert [ki, ko, Te]
blocks, y as [mtile, q, 128, NQW] contiguous blocks (reordered host-side).

Numerics: operands stream as fp16 (1 PE cycle/row, fp32 PSUM accumulation).
Measured absmax/scale error ~3e-4 vs the fp32 reference (fp32r: 1.5e-4 but
2x the input DMA traffic; plain fp32: exact but 4 cycles/row). MODE picks.
"""

import numpy as np

G, N_TOK, D_IN, D_OUT, CAP = 64, 32768, 1024, 4096, 768
M_CORES = 8
EPC = G // M_CORES          # experts per core
P = 128                     # partitions / k-tile / m-tile
KO = D_IN // P              # 8 k-tiles
MODE = "f16"                # "f16" | "f32r" | "f32"
NQW = 2048 if MODE == "f16" else 1024   # n-slab width (SBUF budget bound)
NQ = D_OUT // NQW
NB = NQW // 512             # psum banks per slab

_cache = {}


def _mm_dt(mybir):
    return {
        "f16": mybir.dt.float16,
        "f32r": mybir.dt.float32r,
        "f32": mybir.dt.float32,
    }[MODE]


def _np_dt():
    return np.float16 if MODE == "f16" else np.float32




def _slot_order(mt):
    """Pairwise big-first order: within each adjacent slot pair process the
    bigger expert first. A big expert's long compute window (many m-tiles
    per W byte) lets the W prefetch stream run ahead, so the following
    small expert's burst demand is already buffered."""
    order = []
    for p in range(0, EPC - 1, 2):
        order += sorted((p, p + 1), key=lambda j: -mt[j])
    if EPC % 2:
        order.append(EPC - 1)
    return [j for j in order if mt[j] > 0]

def _build(mt):
    """Compile the SPMD program for per-expert-slot m-tile counts mt (len EPC)."""
    import concourse.mybir as mybir
    import concourse.tile as tile
    from concourse import bacc

    f32 = mybir.dt.float32
    f16 = mybir.dt.float16
    fmm = _mm_dt(mybir)
    n_mtiles = sum(mt)

    nc = bacc.Bacc("TRN2", target_bir_lowering=False, debug=False)
    xt_d = {
        e: nc.dram_tensor(f"xT{e}", [P, KO, P * mt[e]], fmm, kind="ExternalInput")
        for e in range(EPC)
        if mt[e] > 0
    }
    w_d = nc.dram_tensor("W", [EPC, NQ, P, KO, NQW], fmm, kind="ExternalInput")
    y_d = nc.dram_tensor("y", [NQ, P, n_mtiles, NQW], f16, kind="ExternalOutput")
    w_ap, y = w_d.ap(), y_d.ap()

    with tile.TileContext(nc) as tc:
        with (
            tc.tile_pool(name="wq", bufs=4) as wq_pool,
            tc.tile_pool(name="xt", bufs=2) as xt_pool,
            tc.tile_pool(name="st", bufs=3) as st_pool,
            # narrow (laddered) slabs emit small y-writes every ~1.7us, but
            # each small DMA has ~2.5us completion latency (HBM write
            # receipt); a deeper pool keeps the PE from throttling to the
            # y-completion rate
            tc.tile_pool(name="stn", bufs=6) as stn_pool,
            tc.tile_pool(name="wz", bufs=1) as wz_pool,
            tc.tile_pool(name="ps", bufs=8, space="PSUM") as ps_pool,
        ):
            # ~42 warmup matmuls on a zeroed tile: they run during the
            # initial W/xT DMA wait, releasing the PE HAM clock-gate
            # (1.2 -> 2.4 GHz takes ~3.4us of sustained PE activity) so the
            # real MM stream starts warm.
            wz = wz_pool.tile([P, 512], fmm, tag="wz", name="wz")
            nc.vector.memset(wz[:], 0)
            psw = ps_pool.tile([P, 512], f32, tag="ps", name="psw")
            for _ in range(39):
                nc.tensor.matmul(psw[:], wz[:, 0:P], wz[:], start=True, stop=True)

            mi0 = 0  # global m-tile index
            order = _slot_order(mt)
            for ei, e in enumerate(order):
                te = P * mt[e]
                xt = xt_pool.tile([P, KO, te], fmm, tag="xt")
                nc.sync.dma_start(out=xt[:], in_=xt_d[e].ap())
                # ladder-size the first expert's slabs so the first matmul
                # only waits on a 512-wide W transfer
                if NQW >= 2048 and ei == 0:
                    widths = [512, 512, 1024]
                    widths += [NQW] * ((D_OUT - sum(widths)) // NQW)
                else:
                    widths = [NQW] * NQ
                col = 0
                for wi, wd in enumerate(widths):
                    q, ncol, nb = col // NQW, col % NQW, wd // 512
                    wq = wq_pool.tile([P, KO, wd], fmm, tag="wq", name="wq")
                    nc.sync.dma_start(
                        out=wq[:], in_=w_ap[e, q, :, :, ncol : ncol + wd]
                    )
                    get_w = lambda k, nn, wq=wq: wq[
                        :, k, nn * 512 : (nn + 1) * 512
                    ]
                    m = 0
                    while m < mt[e]:
                        # batch two m-tiles per staging tile / y DMA: halves
                        # the y-DMA (and semaphore) count, whose fixed
                        # ~2.5us completion latency otherwise dominates
                        # narrow slabs and stretches the teardown epilogue
                        mb = min(2, mt[e] - m)
                        pool = st_pool if wd >= NQW else stn_pool
                        st = pool.tile([P, mb, wd], f16, tag="st", name="st")
                        for mh in range(mb):
                            pss = [
                                ps_pool.tile([P, 512], f32, tag="ps", name="ps")
                                for _ in range(nb)
                            ]
                            for k in range(KO):
                                lhsT = xt[:, k, (m + mh) * P : (m + mh + 1) * P]
                                for nn in range(nb):
                                    nc.tensor.matmul(
                                        pss[nn][:],
                                        lhsT,
                                        get_w(k, nn),
                                        start=(k == 0),
                                        stop=(k == KO - 1),
                                    )
                            for nn in range(nb):
                                dst = st[:, mh, nn * 512 : (nn + 1) * 512]
                                if nn % 2 == 0:
                                    nc.vector.tensor_copy(dst, pss[nn][:])
                                else:
                                    nc.scalar.copy(dst, pss[nn][:])
                        nc.scalar.dma_start(
                            out=y[q, :, mi0 + m : mi0 + m + mb, ncol : ncol + wd],
                            in_=st[:],
                        )
                        m += mb
                    col += wd
                mi0 += mt[e]
    nc.compile()
    return nc


def _prepare(x, weight, counts):
    """Host-side all-to-all: per-core padded token blocks + weight slices."""
    ndt = _np_dt()
    starts = np.zeros(G + 1, np.int64)
    np.cumsum(counts, out=starts[1:])
    cnt = counts.reshape(M_CORES, EPC)
    mt = tuple(int(v) for v in np.ceil(cnt / P).astype(np.int64).max(axis=0))

    order = _slot_order(mt)
    in_maps, metas = [], []
    for c in range(M_CORES):
        im = {}
        meta = []
        mi0 = 0
        mi0_by_slot = {}
        for j in order:
            mi0_by_slot[j] = mi0
            mi0 += mt[j]
        for j in range(EPC):
            g = c * EPC + j
            s, n = int(starts[g]), int(counts[g])
            n = min(n, N_TOK - s) if s < N_TOK else 0
            if mt[j] == 0:
                continue
            te = P * mt[j]
            xe = np.zeros((te, D_IN), ndt)
            if n > 0:
                xe[:n] = x[s : s + n]
            # [te, D_IN] -> [D_IN, te] -> [KO, P, te] -> [P, KO, te]
            im[f"xT{j}"] = np.ascontiguousarray(
                xe.T.reshape(KO, P, te).transpose(1, 0, 2)
            )
            meta.append((mi0_by_slot[j], s, n))
        # weight [EPC, D_IN, D_OUT] -> [e, q, ki, ko, n]
        wc = weight[c * EPC : (c + 1) * EPC].reshape(EPC, KO, P, NQ, NQW)
        im["W"] = np.ascontiguousarray(wc.transpose(0, 3, 2, 1, 4).astype(ndt))
        in_maps.append(im)
        metas.append(meta)
    return mt, in_maps, metas


def _ensure_axon_hooks_shim():
    """bass_utils imports antenv.axon_hooks when tracing is requested (e.g.
    via a BASS_TRACE env var); some images lack that module. Install a no-op
    shim so the run degrades to untraced instead of crashing."""
    try:
        from antenv.axon_hooks import get_axon_ntff_profile_hook  # noqa: F401
        return
    except ImportError:
        pass
    import sys
    import types

    try:
        import antenv
    except ImportError:
        return
    mod = types.ModuleType("antenv.axon_hooks")
    mod._hook = None
    mod.get_axon_ntff_profile_hook = lambda: getattr(mod, "_hook", None)

    def _set(h):
        mod._hook = h

    mod.set_axon_ntff_profile_hook = _set
    sys.modules["antenv.axon_hooks"] = mod
    antenv.axon_hooks = mod


def _run(x, weight, counts, trace=False, trace_cores=None):
    from concourse.bass_utils import run_bass_kernel_spmd

    _ensure_axon_hooks_shim()

    x = np.ascontiguousarray(np.asarray(x, dtype=np.float32))
    weight = np.ascontiguousarray(np.asarray(weight, dtype=np.float32))
    counts = np.asarray(counts).astype(np.int64)
    assert counts.shape == (G,)

    mt, in_maps, metas = _prepare(x, weight, counts)
    if sum(mt) == 0:
        return np.zeros((N_TOK, D_OUT), np.float32), None
    if mt not in _cache:
        _cache[mt] = _build(mt)
    nc = _cache[mt]

    res = run_bass_kernel_spmd(
        nc,
        in_maps,
        core_ids=list(range(M_CORES)),
        trace=trace,
        trace_cores=trace_cores,
    )
    out = np.zeros((N_TOK, D_OUT), np.float32)
    for c in range(M_CORES):
        yc = np.asarray(res.results[c]["y"], dtype=np.float32)  # [NQ, P, n_mtiles, NQW]
        n_mtiles = yc.shape[2]
        # -> [n_mtiles, P, NQ, NQW] -> [n_mtiles*P, D_OUT]
        yc = yc.transpose(2, 1, 0, 3).reshape(n_mtiles * P, D_OUT)
        for mi0, s, n in metas[c]:
            if n > 0:
                out[s : s + n] = yc[mi0 * P : mi0 * P + n]
    return out, res


def kernel(x, weight, num_inputs_per_group):
    out, _ = _run(x, weight, num_inputs_per_group)
    return out



# revision 20
# speedup vs baseline: 1.1979x; 1.1979x over previous
"""Expert-parallel grouped matmul (MoE BatchLinear) for 8 Trainium2 NeuronCores.

Problem: y[t] = x[t] @ W[g(t)] where tokens are grouped contiguously by expert
g (G=64 experts, counts given at runtime). Sharding: expert-parallel — core c
owns experts [8c, 8c+8) and the contiguous token rows routed to them. The
"all-to-all" is done host-side: kernel() receives full inputs, slices/pads
per-core token blocks, and scatters per-core outputs back.

Device kernel (SPMD, one program on 8 cores):
  for each local expert e (8 per core):
    xT_e resident in SBUF as [128ki, 8ko, Te] (host pre-transposed)
    for each NQW-wide n-slab of W_e (slab [128ki, 8ko, NQW], 4 bufs):
      for each 128-token m-tile:
        8 k-steps x NB matmuls (N=512) accumulate into NB PSUM banks
        DVE copy PSUM -> SBUF staging, DMA staging -> y (scalar-engine ring)

All DRAM layouts are chosen so every DMA reads/writes fully-contiguous
per-partition runs: W as [e, q, ki, ko, n], xT as per-expert [ki, ko, Te]
blocks, y as [mtile, q, 128, NQW] contiguous blocks (reordered host-side).

Numerics: operands stream as fp16 (1 PE cycle/row, fp32 PSUM accumulation).
Measured absmax/scale error ~3e-4 vs the fp32 reference (fp32r: 1.5e-4 but
2x the input DMA traffic; plain fp32: exact but 4 cycles/row). MODE picks.
"""

import numpy as np

G, N_TOK, D_IN, D_OUT, CAP = 64, 32768, 1024, 4096, 768
M_CORES = 8
EPC = G // M_CORES          # experts per core
P = 128                     # partitions / k-tile / m-tile
KO = D_IN // P              # 8 k-tiles
MODE = "f16"                # "f16" | "f32r" | "f32"
NQW = 2048 if MODE == "f16" else 1024   # n-slab width (SBUF budget bound)
NQ = D_OUT // NQW
NB = NQW // 512             # psum banks per slab

_cache = {}


def _mm_dt(mybir):
    return {
        "f16": mybir.dt.float16,
        "f32r": mybir.dt.float32r,
        "f32": mybir.dt.float32,
    }[MODE]


def _np_dt():
    return np.float16 if MODE == "f16" else np.float32




def _slot_order(mt):
    """Pairwise big-first order: within each adjacent slot pair process the
    bigger expert first. A big expert's long compute window (many m-tiles
    per W byte) lets the W prefetch stream run ahead, so the following
    small expert's burst demand is already buffered."""
    order = []
    for p in range(0, EPC - 1, 2):
        order += sorted((p, p + 1), key=lambda j: -mt[j])
    if EPC % 2:
        order.append(EPC - 1)
    return [j for j in order if mt[j] > 0]

def _build(mt):
    """Compile the SPMD program for per-expert-slot m-tile counts mt (len EPC)."""
    import concourse.mybir as mybir
    import concourse.tile as tile
    from concourse import bacc

    f32 = mybir.dt.float32
    f16 = mybir.dt.float16
    fmm = _mm_dt(mybir)
    n_mtiles = sum(mt)

    nc = bacc.Bacc("TRN2", target_bir_lowering=False, debug=False)
    xt_d = {
        e: nc.dram_tensor(f"xT{e}", [P, KO, P * mt[e]], fmm, kind="ExternalInput")
        for e in range(EPC)
        if mt[e] > 0
    }
    w_d = nc.dram_tensor("W", [EPC, NQ, P, KO, NQW], fmm, kind="ExternalInput")
    y_d = nc.dram_tensor("y", [NQ, P, n_mtiles, NQW], f16, kind="ExternalOutput")
    w_ap, y = w_d.ap(), y_d.ap()

    with tile.TileContext(nc) as tc:
        with (
            tc.tile_pool(name="wq", bufs=4) as wq_pool,
            tc.tile_pool(name="xt", bufs=2) as xt_pool,
            tc.tile_pool(name="st", bufs=3) as st_pool,
            # narrow (laddered) slabs emit small y-writes every ~1.7us, but
            # each small DMA has ~2.5us completion latency (HBM write
            # receipt); a deeper pool keeps the PE from throttling to the
            # y-completion rate
            tc.tile_pool(name="stn", bufs=6) as stn_pool,
            tc.tile_pool(name="wz", bufs=1) as wz_pool,
            tc.tile_pool(name="ps", bufs=8, space="PSUM") as ps_pool,
        ):
            # ~42 warmup matmuls on a zeroed tile: they run during the
            # initial W/xT DMA wait, releasing the PE HAM clock-gate
            # (1.2 -> 2.4 GHz takes ~3.4us of sustained PE activity) so the
            # real MM stream starts warm.
            wz = wz_pool.tile([P, 512], fmm, tag="wz", name="wz")
            nc.vector.memset(wz[:], 0)
            psw = ps_pool.tile([P, 512], f32, tag="ps", name="psw")
            for _ in range(39):
                nc.tensor.matmul(psw[:], wz[:, 0:P], wz[:], start=True, stop=True)

            mi0 = 0  # global m-tile index
            order = _slot_order(mt)
            for ei, e in enumerate(order):
                te = P * mt[e]
                xt = xt_pool.tile([P, KO, te], fmm, tag="xt")
                nc.sync.dma_start(out=xt[:], in_=xt_d[e].ap())
                # ladder-size the first expert's slabs so the first matmul
                # only waits on a 512-wide W transfer
                if NQW >= 2048 and ei == 0:
                    widths = [512, 512, 1024]
                    widths += [NQW] * ((D_OUT - sum(widths)) // NQW)
                else:
                    widths = [NQW] * NQ
                col = 0
                for wi, wd in enumerate(widths):
                    q, ncol, nb = col // NQW, col % NQW, wd // 512
                    wq = wq_pool.tile([P, KO, wd], fmm, tag="wq", name="wq")
                    nc.sync.dma_start(
                        out=wq[:], in_=w_ap[e, q, :, :, ncol : ncol + wd]
                    )
                    get_w = lambda k, nn, wq=wq: wq[
                        :, k, nn * 512 : (nn + 1) * 512
                    ]
                    m = 0
                    while m < mt[e]:
                        # batch two m-tiles per staging tile / y DMA: halves
                        # the y-DMA (and semaphore) count, whose fixed
                        # ~2.5us completion latency otherwise dominates
                        # narrow slabs and stretches the teardown epilogue
                        mb = min(2, mt[e] - m)
                        pool = st_pool if wd >= NQW else stn_pool
                        st = pool.tile([P, mb, wd], f16, tag="st", name="st")
                        for mh in range(mb):
                            pss = [
                                ps_pool.tile([P, 512], f32, tag="ps", name="ps")
                                for _ in range(nb)
                            ]
                            for k in range(KO):
                                lhsT = xt[:, k, (m + mh) * P : (m + mh + 1) * P]
                                for nn in range(nb):
                                    nc.tensor.matmul(
                                        pss[nn][:],
                                        lhsT,
                                        get_w(k, nn),
                                        start=(k == 0),
                                        stop=(k == KO - 1),
                                    )
                            for nn in range(nb):
                                dst = st[:, mh, nn * 512 : (nn + 1) * 512]
                                if nn % 2 == 0:
                                    nc.vector.tensor_copy(dst, pss[nn][:])
                                else:
                                    nc.scalar.copy(dst, pss[nn][:])
                        last_dma = (
                            ei == len(order) - 1
                            and wi == len(widths) - 1
                            and m + mb >= mt[e]
                        )
                        # final y write rides the (by now idle) Sync ring:
                        # tests whether the ~10us trigger->completion latency
                        # of the last DMA is a busy-ring artifact
                        dma_eng = nc.sync if last_dma else nc.scalar
                        dma_eng.dma_start(
                            out=y[q, :, mi0 + m : mi0 + m + mb, ncol : ncol + wd],
                            in_=st[:],
                        )
                        m += mb
                    col += wd
                mi0 += mt[e]
    nc.compile()
    return nc


def _prepare(x, weight, counts):
    """Host-side all-to-all: per-core padded token blocks + weight slices."""
    ndt = _np_dt()
    starts = np.zeros(G + 1, np.int64)
    np.cumsum(counts, out=starts[1:])
    cnt = counts.reshape(M_CORES, EPC)
    mt = tuple(int(v) for v in np.ceil(cnt / P).astype(np.int64).max(axis=0))

    order = _slot_order(mt)
    in_maps, metas = [], []
    for c in range(M_CORES):
        im = {}
        meta = []
        mi0 = 0
        mi0_by_slot = {}
        for j in order:
            mi0_by_slot[j] = mi0
            mi0 += mt[j]
        for j in range(EPC):
            g = c * EPC + j
            s, n = int(starts[g]), int(counts[g])
            n = min(n, N_TOK - s) if s < N_TOK else 0
            if mt[j] == 0:
                continue
            te = P * mt[j]
            xe = np.zeros((te, D_IN), ndt)
            if n > 0:
                xe[:n] = x[s : s + n]
            # [te, D_IN] -> [D_IN, te] -> [KO, P, te] -> [P, KO, te]
            im[f"xT{j}"] = np.ascontiguousarray(
                xe.T.reshape(KO, P, te).transpose(1, 0, 2)
            )
            meta.append((mi0_by_slot[j], s, n))
        # weight [EPC, D_IN, D_OUT] -> [e, q, ki, ko, n]
        wc = weight[c * EPC : (c + 1) * EPC].reshape(EPC, KO, P, NQ, NQW)
        im["W"] = np.ascontiguousarray(wc.transpose(0, 3, 2, 1, 4).astype(ndt))
        in_maps.append(im)
        metas.append(meta)
    return mt, in_maps, metas


def _ensure_axon_hooks_shim():
    """bass_utils imports antenv.axon_hooks when tracing is requested (e.g.
    via a BASS_TRACE env var); some images lack that module. Install a no-op
    shim so the run degrades to untraced instead of crashing."""
    try:
        from antenv.axon_hooks import get_axon_ntff_profile_hook  # noqa: F401
        return
    except ImportError:
        pass
    import sys
    import types

    try:
        import antenv
    except ImportError:
        return
    mod = types.ModuleType("antenv.axon_hooks")
    mod._hook = None
    mod.get_axon_ntff_profile_hook = lambda: getattr(mod, "_hook", None)

    def _set(h):
        mod._hook = h

    mod.set_axon_ntff_profile_hook = _set
    sys.modules["antenv.axon_hooks"] = mod
    antenv.axon_hooks = mod


def _run(x, weight, counts, trace=False, trace_cores=None):
    from concourse.bass_utils import run_bass_kernel_spmd

    _ensure_axon_hooks_shim()

    x = np.ascontiguousarray(np.asarray(x, dtype=np.float32))
    weight = np.ascontiguousarray(np.asarray(weight, dtype=np.float32))
    counts = np.asarray(counts).astype(np.int64)
    assert counts.shape == (G,)

    mt, in_maps, metas = _prepare(x, weight, counts)
    if sum(mt) == 0:
        return np.zeros((N_TOK, D_OUT), np.float32), None
    if mt not in _cache:
        _cache[mt] = _build(mt)
    nc = _cache[mt]

    res = run_bass_kernel_spmd(
        nc,
        in_maps,
        core_ids=list(range(M_CORES)),
        trace=trace,
        trace_cores=trace_cores,
    )
    out = np.zeros((N_TOK, D_OUT), np.float32)
    for c in range(M_CORES):
        yc = np.asarray(res.results[c]["y"], dtype=np.float32)  # [NQ, P, n_mtiles, NQW]
        n_mtiles = yc.shape[2]
        # -> [n_mtiles, P, NQ, NQW] -> [n_mtiles*P, D_OUT]
        yc = yc.transpose(2, 1, 0, 3).reshape(n_mtiles * P, D_OUT)
        for mi0, s, n in metas[c]:
            if n > 0:
                out[s : s + n] = yc[mi0 * P : mi0 * P + n]
    return out, res


def kernel(x, weight, num_inputs_per_group):
    out, _ = _run(x, weight, num_inputs_per_group)
    return out

